# revision 93
# baseline (speedup 1.0000x reference)
import sys

sys.path.insert(0, "/opt/trn_rl_repo")

import ml_dtypes
import numpy as np

import concourse.bass as bass
import concourse.tile as tile
from concourse import bacc, mybir
from concourse.bass_utils import run_bass_kernel_spmd

F32 = mybir.dt.float32
F32R = mybir.dt.float32r
BF16 = mybir.dt.bfloat16
AF = mybir.ActivationFunctionType

BATCH = 2
SEQ = 2048
D = 1024
NHEADS = 16
DK = 64
HPC = 4          # heads per core
NCORES = 8
THETA = 10000.0
CHUNK = 512
NCH = SEQ // CHUNK   # 4 chunks of queries
NBLK = SEQ // 128    # 16 key blocks


def _build_nc():
    nc = bacc.Bacc("TRN2", target_bir_lowering=False)
    XT = nc.declare_dram_parameter("XT", [128, 8, SEQ], BF16, isOutput=False)
    WT = nc.declare_dram_parameter("WT", [128, 8, 768], BF16, isOutput=False)
    COS = nc.declare_dram_parameter("COS", [128, SEQ], BF16, isOutput=False)
    SIN = nc.declare_dram_parameter("SIN", [128, SEQ], BF16, isOutput=False)
    WOT = nc.declare_dram_parameter("WOT", [128, 2, D], BF16, isOutput=False)
    INDT = nc.declare_dram_parameter("INDT", [128, 4], BF16, isOutput=False)
    I2 = nc.declare_dram_parameter("I2", [4, 128], BF16, isOutput=False)
    LNG = nc.declare_dram_parameter("LNG", [4, 1], F32, isOutput=False)
    TRI = nc.declare_dram_parameter("TRI", [128, 128], BF16, isOutput=False)
    OUT = nc.declare_dram_parameter("OUT", [SEQ, D], BF16, isOutput=True)

    with tile.TileContext(nc) as tc:
        with (
            nc.allow_low_precision(reason="bf16 matmuls validated at 1e-2 rel err"),
            tc.tile_pool(name="cst", bufs=1) as cst,
            tc.tile_pool(name="xtp", bufs=4) as xtp,
            tc.tile_pool(name="tmp", bufs=10) as tmp,
            tc.tile_pool(name="expp", bufs=14) as expp,
            tc.tile_pool(name="bcp", bufs=4) as bcp,
            tc.tile_pool(name="ocp", bufs=6) as ocp,
            tc.tile_pool(name="ps", bufs=2, space="PSUM") as ps,
        ):
            wt_sb = cst.tile([128, 8, 768], BF16, tag="wt")
            cos_sb = cst.tile([128, SEQ], BF16, tag="cos")
            sin_sb = cst.tile([128, SEQ], BF16, tag="sin")
            wot_sb = cst.tile([128, 2, D], BF16, tag="wot")
            indt_sb = cst.tile([128, 4], BF16, tag="indt")
            i2_sb = cst.tile([4, 128], BF16, tag="i2")
            lng_sb = cst.tile([4, 1], F32, tag="lng")
            tri_sb = cst.tile([128, 128], BF16, tag="tri")
            q_sb = cst.tile([128, 2, SEQ], BF16, tag="q")
            k_sb = cst.tile([128, 2, SEQ], BF16, tag="k")
            v_sb = cst.tile([128, NBLK, HPC, 65], BF16, tag="v")
            ot_sb = cst.tile([128, 2, SEQ], BF16, tag="ot")
            kre_sb = cst.tile([128, NBLK * HPC], F32, tag="kre")
            dum_sb = cst.tile([1, 64], F32, tag="dum")

            xts = {}

            def emit_xt(c):
                c0 = c * CHUNK
                xt_t = xtp.tile([128, 8, CHUNK], BF16, tag="xt", name=f"xt_{c}")
                nc.sync.dma_start(out=xt_t[:, 0:4, :], in_=XT[:, 0:4, c0:c0 + CHUNK])
                nc.sync.dma_start(out=xt_t[:, 4:8, :], in_=XT[:, 4:8, c0:c0 + CHUNK])
                xts[c] = xt_t

            def emit_cs(c):
                c0 = c * CHUNK
                nc.sync.dma_start(out=cos_sb[:, c0:c0 + CHUNK],
                                  in_=COS[:, c0:c0 + CHUNK])
                nc.sync.dma_start(out=sin_sb[:, c0:c0 + CHUNK],
                                  in_=SIN[:, c0:c0 + CHUNK])

            # startup: first proj needs xt(0) di-slices + Q weight columns;
            # spread the first pieces across engine DGE queues (SP / Pool /
            # ACT / DVE run their DMAs concurrently) so the first matmul can
            # start as early as possible
            xt0 = xtp.tile([128, 8, CHUNK], BF16, tag="xt", name="xt_0")
            xts[0] = xt0
            nc.sync.dma_start(out=wt_sb[:, 0:1, 0:128], in_=WT[:, 0:1, 0:128])
            nc.gpsimd.dma_start(out=xt0[:, 0:2, :], in_=XT[:, 0:2, 0:CHUNK])
            nc.scalar.dma_start(out=xt0[:, 2:5, :], in_=XT[:, 2:5, 0:CHUNK])
            nc.sync.dma_start(out=wt_sb[:, 1:8, 0:128], in_=WT[:, 1:8, 0:128])
            nc.gpsimd.dma_start(out=xt0[:, 5:8, :], in_=XT[:, 5:8, 0:CHUNK])
            nc.sync.dma_start(out=wt_sb[:, :, 128:256], in_=WT[:, :, 128:256])
            nc.gpsimd.dma_start(out=wt_sb[:, :, 256:512], in_=WT[:, :, 256:512])
            nc.sync.dma_start(out=cos_sb[:, 0:CHUNK], in_=COS[:, 0:CHUNK])
            nc.sync.dma_start(out=sin_sb[:, 0:CHUNK], in_=SIN[:, 0:CHUNK])
            nc.sync.dma_start(out=indt_sb[:], in_=INDT[:])
            nc.sync.dma_start(out=i2_sb[:], in_=I2[:])
            nc.sync.dma_start(out=lng_sb[:].bitcast(F32R), in_=LNG[:].bitcast(F32R))
            nc.sync.dma_start(out=wt_sb[:, :, 512:768], in_=WT[:, :, 512:768])
            nc.sync.dma_start(out=tri_sb[:], in_=TRI[:])
            emit_xt(1)
            emit_cs(1)
            nc.sync.dma_start(out=wot_sb[:], in_=WOT[:])

            # ones column 64 of each v block for the denominator trick (data
            # cols are overwritten by the V projection); gpsimd keeps it off
            # the DVE queue
            nc.gpsimd.memset(dum_sb[:], 1.0)
            nc.gpsimd.memset(v_sb[:], 1.0)
            # pre-load the combined ln+exp act table so the table-load pass
            # (greedy first-fit per function) never has to swap tables
            nc.scalar.add_instruction(mybir.InstLoadActFuncSet(
                name=nc.get_next_instruction_name(),
                act_func_set_id=6, engine=mybir.EngineType.Activation))
            dln = tmp.tile([1, 64], F32, tag="t", name="dln")
            nc.scalar.activation(dln[:], dum_sb[:], AF.Ln)

            def projqk_units(c, qk, units, tail, act_cp=True):
                c0 = c * CHUNK
                qoff = 256 * qk
                dst = q_sb if qk == 0 else k_sb
                st = {}

                def mk_mm(which, di, qo):
                    def u(which=which, di=di, qo=qo):
                        if di == 0:
                            st[which] = ps.tile(
                                [128, CHUNK], F32, tag="pp",
                                name=f"p{which}_{qk}_{c}")
                        nc.tensor.matmul(
                            st[which],
                            lhsT=wt_sb[:, di, qo:qo + 128],
                            rhs=xts[c][:, di, :],
                            start=(di == 0), stop=(di == 7),
                        )
                    return u

                for di in range(8):
                    units.append(mk_mm("A", di, qoff))

                cp = nc.scalar.copy if act_cp else nc.vector.tensor_copy

                def uA():
                    pAc = tmp.tile([128, CHUNK], BF16, tag="t",
                                   name=f"pAc_{qk}_{c}")
                    cp(pAc[:], st["A"][:])
                    sqA = tmp.tile([128, CHUNK], BF16, tag="t",
                                   name=f"sqA_{qk}_{c}")
                    nc.vector.tensor_mul(sqA[:], pAc[:], pAc[:])
                    st["Ac"], st["sqA"] = pAc, sqA
                units.append(uA)

                for di in range(8):
                    units.append(mk_mm("B", di, qoff + 128))

                def uB():
                    pBc = tmp.tile([128, CHUNK], BF16, tag="t",
                                   name=f"pBc_{qk}_{c}")
                    cp(pBc[:], st["B"][:])
                    sqB = tmp.tile([128, CHUNK], BF16, tag="t",
                                   name=f"sqB_{qk}_{c}")
                    nc.vector.tensor_mul(sqB[:], pBc[:], pBc[:])
                    ssum = tmp.tile([128, CHUNK], BF16, tag="t",
                                    name=f"ssum_{qk}_{c}")
                    nc.vector.tensor_add(ssum[:], st["sqA"][:], sqB[:])
                    st["Bc"], st["ssum"] = pBc, ssum
                units.append(uB)

                cs = cos_sb[:, c0:c0 + CHUNK]
                sn = sin_sb[:, c0:c0 + CHUNK]

                if qk == 0:
                    # per-(head, position) g/||q||: n2 -> exp(-.5 ln + ln g)
                    def uN():
                        n2 = ps.tile([4, CHUNK], F32, tag="mm", name=f"n2_{c}")
                        nc.tensor.matmul(n2, lhsT=indt_sb[:], rhs=st["ssum"][:],
                                         start=True, stop=True)
                        lnq = tmp.tile([4, CHUNK], F32, tag="t", name=f"lnq_{c}")
                        nc.scalar.activation(lnq[:], n2[:], AF.Ln)
                        rbq = tmp.tile([4, CHUNK], BF16, tag="t", name=f"rbq_{c}")
                        nc.scalar.activation(rbq[:], lnq[:], AF.Exp,
                                             bias=lng_sb[:], scale=-0.5)
                        st["rbq"] = rbq
                    tail.append(uN)

                    def uBC():
                        rbp = ps.tile([128, CHUNK], F32, tag="mm", name=f"rbp_{c}")
                        nc.tensor.matmul(rbp, lhsT=i2_sb[:], rhs=st["rbq"][:],
                                         start=True, stop=True)
                        rb = tmp.tile([128, CHUNK], BF16, tag="t", name=f"rb_{c}")
                        cp(rb[:], rbp[:])
                        rbc = tmp.tile([128, CHUNK], BF16, tag="t", name=f"rbc_{c}")
                        nc.vector.tensor_mul(rbc[:], rb[:], cs)
                        rbs = tmp.tile([128, CHUNK], BF16, tag="t", name=f"rbs_{c}")
                        nc.vector.tensor_mul(rbs[:], rb[:], sn)
                        st["rbc"], st["rbs"] = rbc, rbs
                    tail.append(uBC)
                else:
                    # per-key 1/||k||, transposed to [key, head] for use as
                    # the exp() scale operand
                    def uKN():
                        kn = ps.tile([128, 16], F32, tag="mm", name=f"kn_{c}")
                        for bb in range(4):
                            nc.tensor.matmul(
                                kn[:, 4 * bb:4 * bb + 4],
                                lhsT=st["ssum"][:, bb * 128:bb * 128 + 128],
                                rhs=indt_sb[:], start=True, stop=True,
                            )
                        lnk = tmp.tile([128, 16], F32, tag="kt", name=f"lnk_{c}")
                        nc.scalar.activation(lnk[:], kn[:], AF.Ln)
                        nc.scalar.activation(kre_sb[:, 16 * c:16 * c + 16],
                                             lnk[:], AF.Exp, scale=-0.5)
                    tail.append(uKN)

                def uProd():
                    pc = st["rbc"][:] if qk == 0 else cs
                    pss = st["rbs"][:] if qk == 0 else sn
                    for nm, src, mulby in (("tac", "Ac", pc), ("tas", "Ac", pss),
                                           ("tbc", "Bc", pc), ("tbs", "Bc", pss)):
                        t = tmp.tile([128, CHUNK], BF16, tag="t",
                                     name=f"{nm}_{qk}_{c}")
                        nc.vector.tensor_mul(t[:], st[src][:], mulby)
                        st[nm] = t
                tail.append(uProd)

                def uComb():
                    for h in range(HPC):
                        po = (h % 2) * 64
                        ti = h // 2
                        hs = 32 * h
                        nc.vector.tensor_sub(
                            dst[po:po + 32, ti, c0:c0 + CHUNK],
                            st["tac"][hs:hs + 32, :], st["tbs"][hs:hs + 32, :])
                        nc.vector.tensor_add(
                            dst[po + 32:po + 64, ti, c0:c0 + CHUNK],
                            st["tas"][hs:hs + 32, :], st["tbc"][hs:hs + 32, :])
                tail.append(uComb)

            def projv_units(c, units, act_cp=True, copy_units=None, tags=None):
                for bb in range(4):
                    nb = 4 * c + bb
                    st = {}
                    tg = tags[bb] if tags else "mm"

                    def u1(bb=bb, nb=nb, st=st, tg=tg):
                        st["v"] = ps.tile([128, HPC, 64], F32, tag=tg,
                                          name=f"vps_{nb}")
                        for di in range(4):
                            nc.tensor.matmul(
                                st["v"],
                                lhsT=xts[c][:, di, bb * 128:bb * 128 + 128],
                                rhs=wt_sb[:, di, 512:768],
                                start=(di == 0), stop=False,
                            )

                    def ucp(bb=bb, nb=nb, st=st):
                        if act_cp:
                            nc.scalar.copy(v_sb[:, nb, :, 0:64], st["v"][:])
                        else:
                            nc.vector.tensor_copy(v_sb[:, nb, :, 0:64], st["v"][:])

                    def u2(bb=bb, nb=nb, st=st, ucp=ucp):
                        for di in range(4, 8):
                            nc.tensor.matmul(
                                st["v"],
                                lhsT=xts[c][:, di, bb * 128:bb * 128 + 128],
                                rhs=wt_sb[:, di, 512:768],
                                start=False, stop=(di == 7),
                            )
                        if copy_units is None:
                            ucp()

                    units.append(u1)
                    units.append(u2)
                    if copy_units is not None:
                        copy_units.append(ucp)

            def outproj_units(c, units, act_copy=False):
                for bb in range(4):
                    nb = 4 * c + bb
                    for oc in range(2):
                        def u(nb=nb, oc=oc):
                            wo = ps.tile([128, CHUNK], F32, tag="mm",
                                         name=f"wo_{nb}_{oc}")
                            for ti in range(2):
                                nc.tensor.matmul(
                                    wo,
                                    lhsT=ot_sb[:, ti, nb * 128:nb * 128 + 128],
                                    rhs=wot_sb[:, ti, oc * CHUNK:oc * CHUNK + CHUNK],
                                    start=(ti == 0), stop=(ti == 1),
                                )
                            ob = ocp.tile([128, CHUNK], BF16, tag="ob",
                                          name=f"ob_{nb}_{oc}")
                            if act_copy:
                                nc.scalar.copy(ob[:], wo[:])
                            else:
                                nc.vector.tensor_copy(ob[:], wo[:])
                            nc.sync.dma_start(
                                out=OUT[nb * 128:nb * 128 + 128,
                                        oc * CHUNK:oc * CHUNK + CHUNK],
                                in_=ob[:])
                        units.append(u)

            def mk_fill(units, rate=1.0):
                state = {"i": 0, "cr": 0.0}

                def fill(n):
                    state["cr"] += n * rate
                    while state["cr"] >= 1.0 and state["i"] < len(units):
                        units[state["i"]]()
                        state["i"] += 1
                        state["cr"] -= 1.0
                return fill

            def issue_sc(c, h, jb):
                c0 = c * CHUNK
                po = (h % 2) * 64
                ti = h // 2
                bb = jb - 4 * c
                lo = 128 * bb if bb > 0 else 0
                sc = ps.tile([128, CHUNK], F32, tag="sc", bufs=3,
                             name=f"sc_{h}_{c}_{jb}")
                nc.tensor.matmul(
                    sc[:, lo:],
                    lhsT=k_sb[po:po + 64, ti, jb * 128:jb * 128 + 128],
                    rhs=q_sb[po:po + 64, ti, c0 + lo:c0 + CHUNK],
                    start=True, stop=True,
                )
                return sc, lo

            def emit_attn_head(c, h, fill, ot_piece_cb=None, pre=0,
                               pre_sc=None, last=False):
                c0 = c * CHUNK
                njb = 4 * (c + 1)
                po = (h % 2) * 64
                ti = h // 2
                av = ps.tile([65, CHUNK], F32, tag="av", bufs=1,
                             name=f"av_{h}_{c}")

                # stagger: issue sc for jb+1 before draining jb so PE keeps
                # ahead of ACT's exp stream; fill PE bubbles with proj work.
                # the last block pre-issues the NEXT head's first sc so the
                # exp stream never ramps at head boundaries.
                nxt_sc = None
                scq = [pre_sc if pre_sc is not None else issue_sc(c, h, 0)]
                if pre:
                    fill(pre)
                if njb > 1:
                    scq.append(issue_sc(c, h, 1))
                for jb in range(njb):
                    if jb + 2 < njb:
                        scq.append(issue_sc(c, h, jb + 2))
                    elif jb + 2 == njb and not last:
                        nxt_sc = issue_sc(c, h + 1, 0)
                    sc, lo = scq.pop(0)
                    diag = jb >= 4 * c
                    ex = expp.tile([128, CHUNK], BF16, tag="ex",
                                   name=f"ex_{h}_{c}_{jb}")
                    nc.scalar.activation(
                        ex[:, lo:], sc[:, lo:], AF.Exp,
                        scale=kre_sb[:, 4 * jb + h:4 * jb + h + 1])
                    if diag:
                        # zero the strictly-upper triangle of the 128-col
                        # window at the causal boundary (on the idle Pool
                        # engine; gpsimd is SBUF-only and ex is in SBUF)
                        nc.gpsimd.tensor_mul(
                            ex[:, lo:lo + 128], ex[:, lo:lo + 128], tri_sb[:])
                    nc.tensor.matmul(
                        av[:, lo:], lhsT=v_sb[:, jb, h, :], rhs=ex[:, lo:],
                        start=(jb == 0), stop=(jb == njb - 1),
                        skip_group_check=True,
                    )
                    fill(1)

                # drain the accumulator to SBUF in one copy so its PSUM
                # bank frees for the next head ~1.7us earlier than waiting
                # out the recip/broadcast/normalize chain
                avc = bcp.tile([65, CHUNK], F32, tag="avc", name=f"avc_{h}_{c}")
                nc.vector.tensor_copy(avc[:], av[:])
                srec = bcp.tile([1, CHUNK], F32, tag="srec", name=f"srec_{h}_{c}")
                rb2 = bcp.tile([64, CHUNK], F32, tag="rb2", name=f"rb2_{h}_{c}")
                if ot_piece_cb is None:
                    for pp2 in range(2):
                        lo = 256 * pp2
                        nc.vector.reciprocal(
                            srec[:, lo:lo + 256].bitcast(F32R),
                            avc[64:65, lo:lo + 256])
                        nc.gpsimd.partition_broadcast(
                            rb2[:, lo:lo + 256], srec[0:1, lo:lo + 256])
                        nc.vector.tensor_mul(
                            ot_sb[po:po + 64, ti, c0 + lo:c0 + lo + 256],
                            avc[0:64, lo:lo + 256], rb2[:, lo:lo + 256])
                else:
                    # last head of the last chunk: run the recip/broadcast/
                    # ot chain in 128-col pieces so the final output
                    # projection pipelines behind it
                    for p in range(4):
                        lo = 128 * p
                        nc.vector.reciprocal(
                            srec[:, lo:lo + 128].bitcast(F32R),
                            avc[64:65, lo:lo + 128])
                        nc.gpsimd.partition_broadcast(
                            rb2[:, lo:lo + 128], srec[0:1, lo:lo + 128])
                        nc.vector.tensor_mul(
                            ot_sb[po:po + 64, ti, c0 + lo:c0 + lo + 128],
                            avc[0:64, lo:lo + 128], rb2[:, lo:lo + 128])
                        ot_piece_cb(p)
                return nxt_sc

            # warm-up: chunk 0 projections run back-to-back (no attention
            # yet to interleave with); bulk matmuls first so PE has a long
            # runway while the DVE/ACT rope chain catches up, V last so its
            # PE work covers the rope-chain drain
            bulk0, tail0 = [], []
            projqk_units(0, 0, bulk0, tail0, act_cp=True)
            nq = len(tail0)
            projqk_units(0, 1, bulk0, tail0, act_cp=True)
            # K rope/norm first: k_sb(0) is ready early so phase-1 attention
            # is gated only by the longer Q-norm chain
            tail0 = tail0[nq:] + tail0[:nq]
            projv_units(0, tail0, act_cp=True)
            for u in bulk0 + tail0:
                u()

            # steady phases: attention for chunk c-1 interleaved, at matmul
            # granularity, with chunk c's projections and chunk 0's output
            # projection; outproj(1) and outproj(2) are held back as fill
            # for the final (projection-free, exp-bound) phase
            for c in range(1, NCH):
                act_cp = c <= 2
                units, tail = [], []
                if c + 1 < NCH:
                    units.append(lambda c=c: (emit_xt(c + 1), emit_cs(c + 1)))
                if c == 2:
                    outproj_units(0, units)
                projqk_units(c, 0, units, units, act_cp=act_cp)
                vu = []
                projv_units(c, vu, act_cp=True)
                units += vu[:6]
                projqk_units(c, 1, units, units, act_cp=act_cp)
                units += vu[6:]
                blocks = 4 * c * HPC
                credits = blocks + 6 * HPC
                rate = min(2.2, len(units) / credits * (1.25 if c == 1 else 1.0))
                fill = mk_fill(units, rate)
                psc = None
                for h in range(HPC):
                    psc = emit_attn_head(c - 1, h, fill, pre=3 / rate,
                                         pre_sc=psc, last=(h == HPC - 1))
                    fill(4)
                fill(10**6)

            units = []
            outproj_units(NCH - 3, units)
            outproj_units(NCH - 2, units)
            unitsF = []
            outproj_units(NCH - 1, unitsF, act_copy=True)
            fillF = mk_fill(unitsF)
            blocks = 4 * NCH * HPC
            rate = len(units) / (blocks + 6 * HPC) * 1.3
            fill = mk_fill(units, rate)
            psc = None
            for h in range(HPC):
                if h < HPC - 1:
                    psc = emit_attn_head(NCH - 1, h, fill, pre=2 / rate,
                                         pre_sc=psc)
                else:
                    emit_attn_head(NCH - 1, h, fill,
                                   ot_piece_cb=lambda p: fillF(2),
                                   pre=2 / rate, pre_sc=psc, last=True)
                fill(4)
            fill(10**6)
            fillF(10**6)
    return nc


_NC = None


def _get_nc():
    global _NC
    if _NC is None:
        _NC = _build_nc()
        _NC.finalize()
    return _NC


def _shared_tables(token_positions):
    freqs = np.arange(0, DK, 2, dtype=np.float64)
    inv_theta = THETA ** (-freqs / DK)                      # [32]
    pos = token_positions.astype(np.float64)
    ang = inv_theta[:, None] * pos[None, :]                 # [32, SEQ]
    cos_t = np.ascontiguousarray(
        np.tile(np.cos(ang), (4, 1))).astype(ml_dtypes.bfloat16)
    sin_t = np.ascontiguousarray(
        np.tile(np.sin(ang), (4, 1))).astype(ml_dtypes.bfloat16)

    indt = np.zeros((128, 4), dtype=np.float32)
    for j in range(4):
        indt[32 * j:32 * j + 32, j] = 1.0
    i2 = np.ascontiguousarray(indt.T).astype(ml_dtypes.bfloat16)

    p_i = np.arange(128)[:, None]
    t_i = np.arange(128)[None, :]
    tri = (p_i <= t_i).astype(ml_dtypes.bfloat16)
    return cos_t, sin_t, indt.astype(ml_dtypes.bfloat16), i2, tri


def _core_inputs(c, x, W_QKV, W_O, qk_scale, shared):
    cos_t, sin_t, indt, i2, tri = shared
    b = c // 4
    a = c % 4
    heads = [4 * a + i for i in range(HPC)]

    qA = [64 * h + 2 * t for h in heads for t in range(32)]
    qB = [64 * h + 2 * t + 1 for h in heads for t in range(32)]
    kA = [1024 + r for r in qA]
    kB = [1024 + r for r in qB]
    vr = [2048 + 64 * h + j for h in heads for j in range(DK)]
    rows = qA + qB + kA + kB + vr
    wt = np.ascontiguousarray(
        W_QKV[rows, :].T.reshape(8, 128, 768).transpose(1, 0, 2)
    ).astype(ml_dtypes.bfloat16)

    vcols = [64 * h + j for h in heads for j in range(DK)]
    wot = np.ascontiguousarray(
        W_O[:, vcols].T.reshape(2, 128, D).transpose(1, 0, 2)
    ).astype(ml_dtypes.bfloat16)

    xt = np.ascontiguousarray(
        x[b].T.reshape(8, 128, SEQ).transpose(1, 0, 2)
    ).astype(ml_dtypes.bfloat16)

    lng = np.log(qk_scale[heads].astype(np.float64)).astype(
        np.float32).reshape(4, 1)

    return {
        "XT": xt, "WT": wt, "COS": cos_t, "SIN": sin_t, "WOT": wot,
        "INDT": indt, "I2": i2, "LNG": lng, "TRI": tri,
    }


def _run(inputs, trace=False):
    x = np.asarray(inputs["x"], dtype=np.float32)
    token_positions = np.asarray(inputs["token_positions"])
    W_QKV = np.asarray(inputs["W_QKV"], dtype=np.float32)
    W_O = np.asarray(inputs["W_O"], dtype=np.float32)
    qk_scale = np.asarray(inputs["qk_scale"], dtype=np.float32)

    shared = _shared_tables(token_positions)
    nc = _get_nc()
    in_maps = [_core_inputs(c, x, W_QKV, W_O, qk_scale, shared)
               for c in range(NCORES)]
    core_ids = list(range(NCORES))
    kw = {}
    if trace:
        kw = dict(trace=True, trace_cores=core_ids)
    res = run_bass_kernel_spmd(nc, in_maps, core_ids, **kw)
    parts = [np.asarray(r["OUT"], dtype=np.float32) for r in res.results]
    out = np.stack([
        parts[0] + parts[1] + parts[2] + parts[3],
        parts[4] + parts[5] + parts[6] + parts[7],
    ]).astype(np.float32)
    return out, getattr(res, "exec_time_ns", None)


def kernel(**inputs):
    return _run(inputs, trace=False)[0]


def estimate_time_ns():
    from concourse.timeline_sim import TimelineSim
    ts = TimelineSim(_get_nc(), trace=False, no_exec=True)
    return ts.simulate()


def kernel_timed(**inputs):
    out, _ = _run(inputs, trace=False)
    return out, estimate_time_ns()
